# revision 51
# baseline (speedup 1.0000x reference)
"""Distributed causal attention (qkv proj + RoPE + SDPA + out proj) on 8 trn2 cores.

Sharding: data-parallel over batch (B=2), tensor-parallel over heads
(12 heads -> 4 groups of 3). Core c handles batch c//4, heads 3*(c%4)..3*(c%4)+2.
Each core computes a partial output x_b @ Wqkv_heads -> attention -> @ Wo_rows;
the host sums the 4 head-group partials per batch.

Device layout per core (bf16 matmul operands, fp32 PSUM accumulation):
  xT    [768, 2048]  x[b] transposed (C-major), bf16
  wqkv  [768, 576]   columns: [q0 q1 | k0 k1 | q2 k2 | v0 v1 v2] (64 each), bf16
  wo    [192, 768]   Wo rows for the 3 heads, bf16
  cosT/sinT [128, 2048] RoPE tables, bf16 (row r -> head-dim r%64; sinT is
  row-swapped + sign-folded so swap-muls read in0/in1 at the same base partition)
  out   [2048, 768]  fp32 partial (pre-reduction) output

PSUM (8 banks) is one flat layout shared by projection and attention:
  sc tag [128,2,512] x2 bufs (4 banks): q01/k01 proj psum, then score tiles,
    then the final block's Wo psum (double-buffered tail).
  pv tag [128,512] x2 (2 banks): q2k2 proj, v-proj chunks 0-7, PV accumulators.
  wp tag [128,2,512] x1 (2 banks): v-proj chunks 8-15 (pairs), Wo blocks 0-2.
Sharing tags lets the first score matmuls start as soon as the q01/k01 h0
projections drain (~16us earlier than a scoped-pool layout).

Attention is a lag-1 software pipeline over units
  [(01,0), (01,1), (2,0), (01,2), (2,1), (01,3), (2,2), (2,3)]:
PE runs the scores matmuls of unit i+1 while ACT exponentiates unit i, then PE's
PV matmuls of unit i follow. Units (01,j) and (2,0/1) need only the first-half
(T/2) projections, so the h1 projection calls + v-proj are interleaved into the
early units' gaps as filler steps. Softmax normalization: the PV ones-row gives
denominators in PSUM; DVE reciprocal -> gpsimd partition_broadcast -> one fused
DVE multiply writes the normalized bf16 output tile straight from PSUM.
"""
import numpy as np

B, T, C = 2, 2048, 768
H, DH = 12, 64
HPC = 3            # heads per core
NC_ = 8            # cores
QB = 512           # query block
KC = 128           # key chunk
NJ = T // QB       # 4 query blocks
NKC = T // KC      # 16 key chunks
SCALE = 1.0 / float(np.sqrt(DH))
EBIAS = -1.5       # exp bias: keeps exp outputs well inside fp8-e4m3 range
VW = 80            # vones per-(head,chunk) stride: 65 used, padded so the
                   # fp8 DoubleRow weight AP tile step is a multiple of 16

_prog = None


def _build():
    import concourse.bass as bass
    import concourse.tile as tile
    from concourse import bacc, mybir

    f32 = mybir.dt.float32
    bf16 = mybir.dt.bfloat16
    fp8 = mybir.dt.float8e4
    Exp = mybir.ActivationFunctionType.Exp
    DR = mybir.MatmulPerfMode.DoubleRow

    nc = bacc.Bacc("TRN2", target_bir_lowering=False, debug=False)

    xT_p = nc.declare_dram_parameter("xT", [C, T], bf16, isOutput=False)
    wqkv_p = nc.declare_dram_parameter("wqkv", [C, 576], bf16, isOutput=False)
    wo_p = nc.declare_dram_parameter("wo", [HPC * DH, C], bf16, isOutput=False)
    cos_p = nc.declare_dram_parameter("cosT", [128, T], bf16, isOutput=False)
    sin_p = nc.declare_dram_parameter("sinT", [128, T], bf16, isOutput=False)
    out_p = nc.declare_dram_parameter("out", [T, C], bf16, isOutput=True)

    with tile.TileContext(nc) as tc:
        with tc.tile_pool(name="persist", bufs=1) as persist, \
             tc.tile_pool(name="bct", bufs=3) as bcp, \
             tc.tile_pool(name="ostage", bufs=3) as osp, \
             tc.tile_pool(name="scp", bufs=2, space="PSUM") as scp, \
             tc.tile_pool(name="pvp", bufs=2, space="PSUM") as pvp, \
             tc.tile_pool(name="wpp", bufs=1, space="PSUM") as wpp:
            q01 = persist.tile([128, T], bf16, tag="q01")
            k01 = persist.tile([128, T], bf16, tag="k01")
            qk2 = persist.tile([128, T], bf16, tag="qk2")   # rows 0:64 q2, 64:128 q2 dup
            k2al = persist.tile([128, T], bf16, tag="k2al")  # rows 0:64 k2, 64:128 k2 dup
            # v (+ ones column) per (head, chunk), fp8 for DoubleRow PV
            vones = persist.tile([128, HPC, NKC, VW], fp8, tag="vones")
            # bf16 copies of chunks 0-3 for block 0's full-precision PV
            # (block-0 queries attend few keys, so fp8 errors don't average
            # out there; all other blocks use the fp8 DoubleRow path)
            vones0 = persist.tile([128, 4, HPC, DH + 1], bf16, tag="vones0")
            warm = persist.tile([1, 8], f32, tag="warm")
            wpe = persist.tile([1, 16], bf16, tag="wpe")
            warm2 = persist.tile([1, QB], bf16, tag="warm2")
            wq = persist.tile([128, 6, 576], bf16, tag="wq")
            xts = [persist.tile([128, T], bf16, tag=f"xt{k}", name=f"xt{k}")
                   for k in range(6)]
            cosT = persist.tile([128, T], bf16, tag="cosT")
            sinT = persist.tile([128, T], bf16, tag="sinT")
            ropetmp = persist.tile([128, T // 2], bf16, tag="ropetmp")
            # expt[:, hh, c, :] = exp of chunk c for head-slot hh
            expts = [persist.tile([128, 2, NKC, QB], fp8, name=f"expt{i}",
                                  tag=f"expt{i}") for i in range(2)]
            expt0 = persist.tile([128, 2, 4, QB], bf16, tag="expt0")
            outt01 = persist.tile([128, T], bf16, tag="outt01")
            outt2 = persist.tile([64, T], bf16, tag="outt2")
            ebias = persist.tile([128, 1], f32, tag="ebias")
            denom = persist.tile([1, HPC * QB], f32, tag="denom")
            recd = persist.tile([1, HPC * QB], f32, tag="recd")
            wo01 = persist.tile([128, C], bf16, tag="wo01")
            wo2 = persist.tile([64, C], bf16, tag="wo2")

            # preload the exp table set while DMAs run
            nc.vector.memset(warm, 0.0)
            nc.vector.memset(wpe, 0.0)
            nc.vector.memset(warm2, 0.0)
            nc.scalar.activation(out=warm, in_=warm, func=Exp, scale=1.0)
            nc.vector.memset(ebias, EBIAS)
            # vones: zero padding + ones column (fused softmax denominator)
            nc.gpsimd.memset(vones, 0.0)
            nc.gpsimd.memset(vones[:, :, :, DH:DH + 1], 1.0)
            nc.gpsimd.memset(vones0[:, :, :, DH:DH + 1], 1.0)

            # PE warm-up: tiny matmuls during the DMA wait release the
            # HAM clock throttle, then ~2.5us of wide matmuls keep the PE
            # continuously busy past the 3us p-state ramp so the first
            # projection matmuls run at full clock
            wwt = wpp.tile([128, 2, QB], f32, tag="wp", name="warmps")
            wwps = wwt[0:1, 0, 0:16]
            for _ in range(60):
                nc.tensor.matmul(wwps, lhsT=wpe[0:1, 0:1],
                                 rhs=wpe[0:1, :], start=True, stop=True)
            wwps2 = wwt[0:16, 1, :]
            for _ in range(12):
                nc.tensor.matmul(wwps2, lhsT=wpe[0:1, :],
                                 rhs=warm2, start=True, stop=True)

            h0 = slice(0, T // 2)
            h1 = slice(T // 2, T)
            for k in range(6):
                nc.sync.dma_start(
                    out=wq[:, k, :], in_=wqkv_p[k * 128:(k + 1) * 128, :])
                nc.sync.dma_start(out=xts[k][:, h0],
                                  in_=xT_p[k * 128:(k + 1) * 128, h0])
            nc.sync.dma_start(out=cosT, in_=cos_p[:])
            nc.sync.dma_start(out=sinT, in_=sin_p[:])
            for k in range(6):
                nc.sync.dma_start(out=xts[k][:, h1],
                                  in_=xT_p[k * 128:(k + 1) * 128, h1])
            nc.sync.dma_start(out=wo01, in_=wo_p[0:128, :])
            nc.sync.dma_start(out=wo2, in_=wo_p[128:192, :])

            def rope(X, out_q, out_k, sl):
                """RoPE X[:, sl] (2 blocks of 64 rows) in place, except
                that rows 64:128 may go to out_k (for q2k2 -> k2al)."""
                w = sl.stop - sl.start
                tp = ropetmp[:, 0:w]
                nc.vector.tensor_mul(tp[0:32], X[32:64, sl], sinT[32:64, sl])
                nc.vector.tensor_mul(tp[32:64], X[0:32, sl], sinT[0:32, sl])
                nc.vector.tensor_mul(tp[64:96], X[96:128, sl], sinT[96:128, sl])
                nc.vector.tensor_mul(tp[96:128], X[64:96, sl], sinT[64:96, sl])
                nc.vector.tensor_mul(X[:, sl], X[:, sl], cosT[:, sl])
                if out_k is None:
                    nc.vector.tensor_add(X[:, sl], X[:, sl], tp)
                else:
                    nc.vector.tensor_add(out_q[0:64, sl], X[0:64, sl], tp[0:64])
                    nc.vector.tensor_add(out_k[0:64, sl], X[64:128, sl], tp[64:128])

            # q/k projection: M-tile m of qkvT = wqkv cols [128m, 128m+128).
            # rope_blocks selects which QB-blocks of the half to rope now
            # (deferred blocks are roped by filler steps, so the block-0
            # ropes of q01 and k01 run back-to-back on DVE and the first
            # score matmul unblocks ~5us earlier)
            def proj_qk(m, X, out_q=None, out_k=None, half=0,
                        rope_blocks=(0, 1)):
                if m < 2:
                    ps = scp.tile([128, 2, QB], f32, tag="sc",
                                  name=f"pp{m}_{half}")
                    pst = [ps[:, 0, :], ps[:, 1, :]]
                else:
                    pst = [pvp.tile([128, QB], f32, tag="pv",
                                    name=f"pp2_{half}_{nn}") for nn in range(2)]
                # k rotated so the last accumulation step consumes the
                # EARLIEST-arriving x chunk: the call completes right after
                # the k=5 input DMA lands instead of queueing behind it
                for ki, k in enumerate([1, 2, 3, 4, 5, 0]):
                    for nn in range(2):
                        n = half * 2 + nn
                        nc.tensor.matmul(
                            pst[nn],
                            lhsT=wq[:, k, m * 128:(m + 1) * 128],
                            rhs=xts[k][:, n * QB:(n + 1) * QB],
                            start=(ki == 0), stop=(ki == 5))
                for nn in range(2):
                    n = half * 2 + nn
                    nc.scalar.copy(X[:, n * QB:(n + 1) * QB], pst[nn])
                for nn in rope_blocks:
                    n = half * 2 + nn
                    rope(X, out_q, out_k, slice(n * QB, (n + 1) * QB))

            def vproj_pv(t):
                """v-projection key chunk t via a pv psum tile"""
                ps = pvp.tile([128, QB], f32, tag="pv", name=f"vps{t}")
                for k in range(6):
                    nc.tensor.matmul(
                        ps[:, 0:192], lhsT=xts[k][:, t * 128:(t + 1) * 128],
                        rhs=wq[:, k, 384:576],
                        start=(k == 0), stop=(k == 5))
                nc.scalar.copy(
                    vones[:, :, t, 0:DH],
                    ps[:, 0:192].rearrange("p (h d) -> p h d", h=HPC))
                if t < 4:
                    nc.scalar.copy(
                        vones0[:, t, :, 0:DH],
                        ps[:, 0:192].rearrange("p (h d) -> p h d", h=HPC))

            def vproj_wp(t0):
                """v-projection key chunks t0, t0+1 via one wp psum tile"""
                pw = wpp.tile([128, 2, QB], f32, tag="wp", name=f"vpl{t0}")
                for tt in range(2):
                    t = t0 + tt
                    for k in range(6):
                        nc.tensor.matmul(
                            pw[:, tt, 0:192],
                            lhsT=xts[k][:, t * 128:(t + 1) * 128],
                            rhs=wq[:, k, 384:576],
                            start=(k == 0), stop=(k == 5))
                for tt in range(2):
                    t = t0 + tt
                    nc.scalar.copy(
                        vones[:, :, t, 0:DH],
                        pw[:, tt, 0:192].rearrange("p (h d) -> p h d", h=HPC))

            def dup(half):
                # duplicate q2/k2 into rows 64:128 so head-2 score matmuls can
                # alternate PE row halves (LDWEIGHTS/matmul overlap)
                sl = h0 if half == 0 else h1
                nc.vector.tensor_copy(qk2[64:128, sl], qk2[0:64, sl])
                nc.vector.tensor_copy(k2al[64:128, sl], k2al[0:64, sl])

            # first projections: q01/k01 block 0 -> unit (01,0) can start;
            # their block-1 ropes are deferred into unit 0's filler gaps
            proj_qk(0, q01, half=0)
            proj_qk(1, k01, half=0)

            def tgt_of(h):
                return (outt01[0:64] if h == 0
                        else (outt01[64:128] if h == 1 else outt2[0:64]))

            def csel(ap, base=0, width=KC):
                """zero the strict upper triangle of a diagonal chunk region
                in place (fp8, on the idle gpsimd engine)"""
                nc.gpsimd.affine_select(
                    out=ap, in_=ap,
                    compare_op=mybir.AluOpType.is_ge, fill=0.0, base=base,
                    pattern=[[1, width]], channel_multiplier=-1)

            def s_steps(unit, expt):
                """Closures: one per 2-bank scores psum group (2 MMs + exp
                [+ gpsimd causal select / pair-edge memset, for diagonal
                groups — PV DoubleRow pairs read both chunks from the pair's
                base offset])."""
                hh, j = unit
                ex = expt0 if j == 0 else expt
                steps = []
                if hh == "01":
                    # heads 0+1 row-packed: per sc tile, 1 chunk each
                    def grp01(c):
                        u = c - 4 * j
                        off = KC * u if u > 0 else 0
                        tqsl = slice(j * QB + off, (j + 1) * QB)
                        sc = scp.tile([128, 2, QB], f32, tag="sc",
                                      name=f"sc01_{j}_{c}")
                        # h0 rows 0:63, h1 rows 64:127
                        nc.tensor.matmul(
                            sc[:, 0, off:QB],
                            lhsT=k01[0:64, c * KC:(c + 1) * KC],
                            rhs=q01[0:64, tqsl],
                            start=True, stop=True)
                        nc.tensor.matmul(
                            sc[:, 1, off:QB],
                            lhsT=k01[64:128, c * KC:(c + 1) * KC],
                            rhs=q01[64:128, tqsl],
                            start=True, stop=True)
                        nc.scalar.activation(
                            out=ex[:, :, c, off:QB],
                            in_=sc[:, :, off:QB],
                            func=Exp, scale=SCALE, bias=ebias)
                        if u >= 0:
                            # one strided select masks both head-slots (the
                            # leading 0-step pattern dim ignores hh)
                            nc.gpsimd.affine_select(
                                out=ex[:, :, c, off:off + KC],
                                in_=ex[:, :, c, off:off + KC],
                                compare_op=mybir.AluOpType.is_ge, fill=0.0,
                                base=0, pattern=[[0, 2], [1, KC]],
                                channel_multiplier=-1)
                            if j > 0 and u % 2 == 1:
                                # pair-edge: chunk c is the odd half of a
                                # PV DoubleRow pair starting at off - KC
                                nc.gpsimd.memset(
                                    ex[:, :, c, off - KC:off], 0.0)
                    for c in range(4 * (j + 1)):
                        steps.append(lambda c=c: grp01(c))
                else:
                    # head 2: alternate row halves for LDW/MM overlap
                    def grp2(g):
                        c0 = 2 * g
                        u0 = c0 - 4 * j
                        off = KC * u0 if u0 > 0 else 0
                        tqsl = slice(j * QB + off, (j + 1) * QB)
                        sc = scp.tile([128, 2, QB], f32, tag="sc",
                                      name=f"sc2_{j}_{g}")
                        for uu in range(2):
                            c = c0 + uu
                            lo = c % 2 == 0
                            nc.tensor.matmul(
                                sc[:, uu, off:QB],
                                lhsT=k2al[0:64, c * KC:(c + 1) * KC] if lo
                                else k2al[64:128, c * KC:(c + 1) * KC],
                                rhs=qk2[0:64, tqsl] if lo else qk2[64:128, tqsl],
                                start=True, stop=True)
                        nc.scalar.activation(
                            out=ex[:, 0, c0:c0 + 2, off:QB],
                            in_=sc[:, :, off:QB],
                            func=Exp, scale=SCALE, bias=ebias)
                        if u0 >= 0:
                            csel(ex[:, 0, c0, off:off + KC])
                            csel(ex[:, 0, c0 + 1, off:off + 2 * KC],
                                 base=-KC, width=2 * KC)
                    for g in range(2 * (j + 1)):
                        steps.append(lambda g=g: grp2(g))
                return steps

            def p_steps(unit, expt):
                """Closures: PV matmul chunk-steps, then per-head softmax
                normalization (reciprocal -> gpsimd partition-broadcast ->
                fused normalize+copy out of PSUM), then (for '2' units) the
                block's output projection."""
                lastu = unit == ("2", 3)
                hh, j = unit
                nch = 4 * (j + 1)
                heads = [(0, 0), (1, 1)] if hh == "01" else [(2, 0)]
                pos = {}
                bcts = {}
                steps = []

                def setup():
                    for h, _ in heads:
                        pos[h] = pvp.tile([128, QB], f32, tag="pv",
                                          name=f"po_{h}_{j}")

                def chunkpair(c0):
                    # fp8 DoubleRow: one matmul contracts chunks c0 and c0+1
                    u0 = c0 - 4 * j
                    off = KC * u0 if u0 > 0 else 0
                    for h, hh_slot in heads:
                        nc.tensor.matmul(
                            pos[h][0:DH + 1, off:QB],
                            lhsT=vones[:, h, c0:c0 + 2, 0:DH + 1],
                            rhs=expt[:, hh_slot, c0:c0 + 2, off:QB],
                            start=(c0 == 0), stop=(c0 == nch - 2),
                            perf_mode=DR, skip_group_check=True)

                def chunk0(c):
                    # block 0: classic bf16 PV at full precision
                    off = KC * c if c > 0 else 0
                    for h, hh_slot in heads:
                        nc.tensor.matmul(
                            pos[h][0:DH + 1, off:QB],
                            lhsT=vones0[:, c, h, :],
                            rhs=expt0[:, hh_slot, c, off:QB],
                            start=(c == 0), stop=(c == 3),
                            skip_group_check=True)

                steps.append(setup)
                if j == 0:
                    for c0 in range(0, nch, 2):
                        def two0(c0=c0):
                            chunk0(c0)
                            chunk0(c0 + 1)
                        steps.append(two0)
                else:
                    for c0 in range(0, nch, 2):
                        steps.append(lambda c0=c0: chunkpair(c0))

                def fin_h(h):
                    po = pos[h]
                    dcp = nc.scalar.copy if lastu else nc.vector.tensor_copy
                    dcp(denom[0:1, h * QB:(h + 1) * QB],
                        po[DH:DH + 1, :])
                    if h == 1:
                        # h1's target sits at base partition 64; a shifted
                        # PSUM-read multiply is unverified on hw, so keep the
                        # copy + in-place multiply for this head
                        nc.vector.tensor_copy(
                            tgt_of(h)[:, j * QB:(j + 1) * QB], po[0:DH, :])

                def norm_h(h):
                    with nc.allow_low_precision(
                            reason="softmax denom reciprocal: 18-bit approx"):
                        nc.vector.reciprocal_approx_fast(
                            out=recd[0:1, h * QB:(h + 1) * QB],
                            in_=denom[0:1, h * QB:(h + 1) * QB])
                    # the mul needs bct at the target's base partition (DVE
                    # tensor_tensor requires equal input bases) but
                    # partition_broadcast only writes correctly from base 0 —
                    # so for head 1 broadcast to all 128 partitions and read
                    # the upper half
                    base = 64 if h == 1 else 0
                    bct = bcp.tile([128, QB], f32, tag="bct",
                                   name=f"bct_{h}_{j}")
                    nc.gpsimd.partition_broadcast(
                        bct[0:base + 64, :], recd[0:1, h * QB:(h + 1) * QB])
                    sl = slice(j * QB, (j + 1) * QB)
                    tgt = tgt_of(h)
                    if h == 1:
                        nc.vector.tensor_mul(tgt[:, sl], tgt[:, sl],
                                             bct[base:base + 64, :])
                    else:
                        # fused normalize+drain: one DVE pass writes the
                        # normalized bf16 output straight from PSUM
                        # (all operands at base partition 0)
                        nc.vector.tensor_mul(tgt[:, sl], pos[h][0:DH, :],
                                             bct[0:64, :])

                for h, _ in heads:
                    steps.append(lambda h=h: fin_h(h))
                for h, _ in heads:
                    steps.append(lambda h=h: norm_h(h))

                if hh == "2":
                    pws = {}

                    def wo_mm(qq):
                        q = j * 4 + qq
                        if lastu:
                            pw = scp.tile([128, 2, QB], f32, tag="sc",
                                          name=f"pwl_{q}")
                        else:
                            pw = wpp.tile([128, 2, QB], f32, tag="wp",
                                          name=f"pw_{q}")
                        pwA, pwB = pw[:, 0, :], pw[:, 1, 0:256]
                        pws[qq] = (pwA, pwB)
                        for dst, (n0, n1) in ((pwA, (0, 512)),
                                              (pwB, (512, 768))):
                            nc.tensor.matmul(
                                dst,
                                lhsT=outt01[:, q * 128:(q + 1) * 128],
                                rhs=wo01[:, n0:n1],
                                start=True, stop=False)
                            nc.tensor.matmul(
                                dst,
                                lhsT=outt2[:, q * 128:(q + 1) * 128],
                                rhs=wo2[:, n0:n1],
                                start=False, stop=True)

                    def wo_out(qq):
                        q = j * 4 + qq
                        pwA, pwB = pws[qq]
                        ot = osp.tile([128, C], bf16, tag="ot",
                                      name=f"ot_{q}")
                        if lastu:
                            # tail: split the two casts across ACT and DVE so
                            # the psum slot frees in one copy-time
                            nc.scalar.copy(ot[:, 0:QB], pwA)
                            nc.vector.tensor_copy(ot[:, QB:C], pwB)
                        else:
                            nc.vector.tensor_copy(ot[:, 0:QB], pwA)
                            nc.vector.tensor_copy(ot[:, QB:C], pwB)
                        nc.sync.dma_start(
                            out=out_p[q * 128:(q + 1) * 128, :], in_=ot)
                    for qq in range(4):
                        f1 = lambda qq=qq: wo_mm(qq)
                        f1.is_wo = True
                        f2 = lambda qq=qq: wo_out(qq)
                        f2.is_wo = True
                        steps.append(f1)
                        steps.append(f2)
                return steps

            # (2,2) before (01,3) so block 2's output projection hides under
            # (01,3)'s 16 exp groups instead of piling into the tail
            units = [("01", 0), ("01", 1), ("2", 0), ("01", 2),
                     ("2", 1), ("2", 2), ("01", 3), ("2", 3)]

            # filler steps: remaining projections + v-proj, interleaved into
            # the early units' exp-paced gaps (deadlines in unit indices:
            # m0h1/m1h1 by idx3, m2h0+dup0 by idx2, m2h1+dup1 by idx6,
            # v0-3 by idx1, v4-7 by idx2, v8-11 by idx4, v12-15 by idx6)
            # deadlines (unit idx): q01 h1-b0 by idx2 grp0, k01 h1-b0 by
            # idx2 grp8, h1-b1 by idx6, m2h0+dup0 by idx3, m2h1+dup1 by
            # idx5, v0-3 by idx1's prev_p, v4-7 by idx2's, v8-11 by idx3's,
            # v12-15 by idx7's
            fill = {
                0: [lambda: proj_qk(2, qk2, out_q=qk2, out_k=k2al, half=0),
                    lambda: dup(0)],
                1: [lambda: vproj_pv(0), lambda: vproj_pv(1),
                    lambda: vproj_pv(2), lambda: vproj_pv(3),
                    lambda: proj_qk(0, q01, half=1)],
                2: [lambda: vproj_pv(4), lambda: vproj_pv(5),
                    lambda: vproj_pv(6), lambda: vproj_pv(7),
                    lambda: proj_qk(1, k01, half=1)],
                3: [lambda: proj_qk(2, qk2, out_q=qk2, out_k=k2al, half=1),
                    lambda: dup(1),
                    lambda: vproj_wp(8), lambda: vproj_wp(10)],
                4: [lambda: vproj_wp(12), lambda: vproj_wp(14)],
            }

            # lag-1 pipeline, interleaved at step granularity: PE runs
            # fillers + the previous unit's PV/Wo steps in the gaps between
            # this unit's score groups (which are paced by ACT's exp).
            prev_p = []
            carry = []
            for i, u in enumerate(units):
                last = i == len(units) - 1
                S = s_steps(u, expts[i % 2])
                work = carry + list(fill.get(i, [])) + prev_p
                carry = []
                if last:
                    # fold the final unit's own PV steps in behind its
                    # score groups (lag 2) so they don't pile up after
                    # the last exp: own chunk-step k needs exp group k.
                    own = p_steps(u, expts[i % 2])
                done = 0
                own_done = 0
                for gi, s in enumerate(S):
                    s()
                    want = ((gi + 1) * len(work)) // len(S)
                    while done < want:
                        work[done]()
                        done += 1
                    if last and gi >= 2:
                        # own[0] is setup; chunk-step k is own[1+k]
                        while own_done < min(gi - 1, len(S) - 1) + 1:
                            own[own_done]()
                            own_done += 1
                # flush non-Wo leftovers (PV/fins must not cross the next
                # unit's exp overwrites); trailing Wo steps spill into the
                # next unit's exp-paced gaps instead of bunching here
                while done < len(work):
                    if last or not getattr(work[done], "is_wo", False):
                        work[done]()
                        done += 1
                    else:
                        carry = work[done:]
                        break
                if last:
                    prev_p = own[own_done:]
                else:
                    prev_p = p_steps(u, expts[i % 2])
            for p in carry:
                p()
            for p in prev_p:
                p()

    nc.compile()
    return nc


def _host_prep(x, Wqkv, Wo, seq_len):
    import ml_dtypes
    bf16 = ml_dtypes.bfloat16
    x = np.asarray(x, dtype=np.float32)
    Wqkv = np.asarray(Wqkv, dtype=np.float32)
    Wo = np.asarray(Wo, dtype=np.float32)
    off = int(np.asarray(seq_len).reshape(()))

    inv = 1.0 / (10000.0 ** (np.arange(0, DH, 2, dtype=np.float64) / DH))  # [32]
    pos = np.arange(T, dtype=np.float64) + off
    ang = pos[:, None] * inv[None, :]                 # [T, 32]
    cs = np.cos(ang).T                                # [32, T]
    sn = np.sin(ang).T
    cos128 = np.empty((128, T), np.float32)
    sin128 = np.empty((128, T), np.float32)
    for blk in range(2):
        r0 = blk * 64
        cos128[r0:r0 + 32] = cs
        cos128[r0 + 32:r0 + 64] = cs
        # row-swapped + sign-folded: row s holds the coefficient X[s] is
        # multiplied by when producing output row s^32 (see rope()).
        sin128[r0:r0 + 32] = sn
        sin128[r0 + 32:r0 + 64] = -sn

    in_maps = []
    for core in range(NC_):
        b, g = core // 4, core % 4
        hs = [3 * g, 3 * g + 1, 3 * g + 2]
        q = [Wqkv[:, h * DH:(h + 1) * DH] for h in hs]
        k = [Wqkv[:, C + h * DH:C + (h + 1) * DH] for h in hs]
        v = [Wqkv[:, 2 * C + h * DH:2 * C + (h + 1) * DH] for h in hs]
        wqkv_l = np.concatenate(
            [q[0], q[1], k[0], k[1], q[2], k[2], v[0], v[1], v[2]], axis=1)
        in_maps.append({
            "xT": np.ascontiguousarray(x[b].T).astype(bf16),
            "wqkv": np.ascontiguousarray(wqkv_l).astype(bf16),
            "wo": np.ascontiguousarray(
                Wo[g * HPC * DH:(g + 1) * HPC * DH, :]).astype(bf16),
            "cosT": cos128.astype(bf16),
            "sinT": sin128.astype(bf16),
        })
    return in_maps


def _run(in_maps, trace=False):
    global _prog
    from concourse.bass_utils import run_bass_kernel_spmd
    if _prog is None:
        _prog = _build()
    return run_bass_kernel_spmd(_prog, in_maps, list(range(NC_)), trace=trace)


def kernel(x, Wqkv, Wo, seq_len):
    in_maps = _host_prep(x, Wqkv, Wo, seq_len)
    res = _run(in_maps, trace=False)
    out = np.zeros((B, T, C), dtype=np.float32)
    for core in range(NC_):
        out[core // 4] += res.results[core]["out"].astype(np.float32)
    return out
